# revision 5
# baseline (speedup 1.0000x reference)
"""CostVolumeLayer3D Trainium2 kernel v5: PE-Gram with per-piy column tiling.

v4 -> v5: each brick's Gram runs as FOUR concurrent col-tile matmuls
(tile_position (0, 32*piy), K=128 block-diag over batch, M=32), each
streaming only its piy's 5-row y-window (24 cells x 8 d = 192 columns).
The y-shift index becomes partition-uniform, so the dumped gram shrinks
384 -> 192 columns (25 -> 12.6 MB), halving copy and dump cost. Inputs
are issued via the idle GPSIMD's SWDGE so dumps never queue behind them.
"""

from contextlib import ExitStack

import numpy as np

_B, _C, _D, _H, _W = 2, 64, 32, 64, 64
_R = 2
_NCH = 125
_NCORES = 8
_DL = _D // _NCORES          # 4
_BK = 4
_NBY = _H // _BK             # 16
_NBX = _W // _BK             # 16
_NBLK = _NBY * _NBX          # 256
_DH = _DL + 2 * _R           # 8
_HP = _H + 2 * _R            # 68
_WP = _W + 2 * _R            # 68
_NA = 5 * 4 * _DH            # 160 region-A cols per piy (yy0 5, xrel 4, d 8)
_NB = 4 * _DH                # 32 region-B cols per piy (xrel 4, d 8)
_NQ = _NA + _NB              # 192
_GRP = 8                     # bricks per dump DMA
_NSTR = 4                    # input stripes
_SY = 20


def _shift_table():
    shifts = []
    for sd in range(-4, 5):
        i = min(2, sd + 2)
        j = sd - i
        for h in range(-2, 3):
            shifts.append((((5 * sd + h) % _NCH), _R - h, _R - i, _R - j))
    return shifts


_SHIFTS = _shift_table()
_NS = len(_SHIFTS)

_prog = None


def _build_program():
    global _prog
    if _prog is not None:
        return _prog

    import concourse.bacc as bacc
    import concourse.mybir as mybir
    import concourse.tile as tile

    f16 = mybir.dt.float16
    f32 = mybir.dt.float32
    nc = bacc.Bacc(trn_type="TRN2", debug=False)
    # block-diag lhsT per (brick, piy): [blk, piy, (b c), m=b*16+pd*4+px]
    x1_d = nc.dram_tensor("x1c", [128, _NBLK, _BK, 32], f16, kind="ExternalInput")
    x2_d = nc.dram_tensor(
        "x2h", [_NSTR, _B * _C, _SY, _WP, _DH], f16, kind="ExternalInput"
    )
    g_d = nc.dram_tensor("gram", [128, _NBLK, _NQ], f16, kind="ExternalOutput")

    with tile.TileContext(nc) as tc:
        with ExitStack() as ctx:
            x2p = ctx.enter_context(tc.tile_pool(name="x2", bufs=1))
            x1p = ctx.enter_context(tc.tile_pool(name="x1", bufs=1))
            psump = ctx.enter_context(tc.tile_pool(name="ps", bufs=8, space="PSUM"))
            stagep = ctx.enter_context(tc.tile_pool(name="st", bufs=5))

            x2s = []
            x1s = []
            nb4 = _NBLK // _NSTR
            for g in range(_NSTR):
                # all inputs ordered on the HWDGE sync queue: stripe 0
                # loads first at full bandwidth; gram dumps go out via
                # GPSIMD SWDGE so they never queue behind the inputs.
                eng = nc.sync
                x2t = x2p.tile([128, _SY, _WP, _DH], f16, tag=f"x2s{g}", name=f"x2s{g}")
                eng.dma_start(x2t[:], x2_d.ap()[g])
                x2s.append(x2t)
                x1t = x1p.tile([128, nb4, _BK, 32], f16, tag=f"x1s{g}", name=f"x1s{g}")
                eng.dma_start(x1t[:], x1_d.ap()[:, g * nb4 : (g + 1) * nb4])
                x1s.append(x1t)

            for grp in range(_NBLK // _GRP):
                st = stagep.tile([128, _GRP, _NQ], f16, tag="st")
                for g2 in range(_GRP // 2):
                    ps = psump.tile([128, 2, _NQ], f32, tag="ps")
                    for bi2 in range(2):
                        blk = grp * _GRP + g2 * 2 + bi2
                        yi, xi = divmod(blk, _NBX)
                        stripe = yi // 4
                        x2t = x2s[stripe]
                        x1t = x1s[stripe]
                        ly0 = _BK * yi - 16 * stripe
                        lblk = blk % nb4
                        x0 = _BK * xi
                        for py in range(_BK):
                            lhs = x1t[:, lblk, py, :]
                            rhs_a = x2t[:, ly0 + py : ly0 + py + 5, x0 + 4 : x0 + 8, :]
                            rhs_b = x2t[:, ly0 + py, x0 : x0 + 4, :]
                            nc.tensor.matmul(
                                ps[32 * py : 32 * py + 32, bi2, 0:_NA],
                                lhsT=lhs,
                                rhs=rhs_a,
                                start=True,
                                stop=True,
                                tile_position=(0, 32 * py),
                            )
                            nc.tensor.matmul(
                                ps[32 * py : 32 * py + 32, bi2, _NA:_NQ],
                                lhsT=lhs,
                                rhs=rhs_b,
                                start=True,
                                stop=True,
                                tile_position=(0, 32 * py),
                            )
                    b0 = g2 * 2
                    if g2 % 2 == 0:
                        nc.vector.tensor_copy(st[:, b0 : b0 + 2, :], ps[:])
                    else:
                        nc.scalar.copy(st[:, b0 : b0 + 2, :], ps[:])
                # tail groups dump via the (by-then idle) sync HWDGE queue:
                # faster completion chain than SWDGE at the kernel tail.
                deng = nc.sync if grp >= 16 else nc.gpsimd
                deng.dma_start(
                    g_d.ap()[:, grp * _GRP : (grp + 1) * _GRP, :], st[:]
                )
    nc.compile()
    _prog = nc
    return nc


def _shard_inputs(x1, x2):
    x1f = (np.asarray(x1, np.float32) * (1.0 / _NCH)).astype(np.float16)
    x2f = np.asarray(x2, np.float32).astype(np.float16)
    x2pad = np.pad(x2f, ((0, 0), (0, 0), (_R, _R), (_R, _R), (_R, _R)))
    in_maps = []
    for k in range(_NCORES):
        d0 = k * _DL
        slab = x1f[:, :, d0 : d0 + _DL]             # [B,C,4,64,64]
        # -> [blk, piy, (b c), b*16+pd*4+px] block-diag over b
        x1c = np.zeros((128, _NBLK, _BK, 32), np.float16)
        r = slab.reshape(_B, _C, _BK, _NBY, _BK, _NBX, _BK)  # b c pd yi py xi px
        r = r.transpose(0, 1, 3, 5, 4, 2, 6)          # b c yi xi py pd px
        r = r.reshape(_B, _C, _NBLK, _BK, _BK * _BK)  # b c blk py (pd px)
        for b in range(_B):
            x1c[b * 64 : (b + 1) * 64, :, :, b * 16 : (b + 1) * 16] = r[b]
        x2h = np.ascontiguousarray(
            x2pad[:, :, d0 : d0 + _DH].transpose(0, 1, 3, 4, 2)
        ).reshape(_B * _C, _HP, _WP, _DH)
        x2str = np.stack([x2h[:, 16 * g : 16 * g + _SY] for g in range(_NSTR)])
        in_maps.append(
            {
                "x1c": np.ascontiguousarray(x1c),
                "x2h": np.ascontiguousarray(x2str),
            }
        )
    return in_maps


_IDX_CACHE = None


def _gather_indices():
    global _IDX_CACHE
    if _IDX_CACHE is not None:
        return _IDX_CACHE
    d = np.arange(_D)
    y = np.arange(_H)
    x = np.arange(_W)
    core = (d // _DL)[:, None, None]
    blk = (y // _BK)[:, None] * _NBX + (x // _BK)[None, :]
    pd = (d % _BK)[:, None, None]
    py = (y % _BK)[None, :, None]
    px = (x % _BK)[None, None, :]
    # partition row: 32*py + b*16 + pd*4 + px  (b added in _gather)
    m_pos = 32 * py + pd * 4 + px
    qs = np.empty((_NS, _D, _H, _W), np.int64)
    chans = np.empty(_NS, np.int64)
    for s, (ch, dd0, yy0, xx0) in enumerate(_SHIFTS):
        chans[s] = ch
        dpp = pd + dd0                                # [32,1,1]
        if xx0 == 4:
            q = (yy0 * 4 + px) * _DH + dpp            # region A, xrel=px
        else:  # yy0 == 0
            xpp = px + xx0
            q = np.where(
                xpp >= 4,
                ((xpp - 4)) * _DH + dpp,              # A with yy0=0
                _NA + xpp * _DH + dpp,                # region B
            )
        qs[s] = np.broadcast_to(q, (_D, _H, _W))
    _IDX_CACHE = (core, blk, m_pos, qs, chans)
    return _IDX_CACHE


def _gather(results):
    core, blk, m_pos, qs, chans = _gather_indices()
    gram = np.stack([np.asarray(results[k]["gram"]) for k in range(_NCORES)])
    out = np.zeros((_B, _NCH, _D, _H, _W), np.float32)
    for b in range(_B):
        m = m_pos + b * 16
        vals = gram[core[None], m[None], blk[None, None], qs].astype(np.float32)
        out[b, chans] = vals
    return out


def _run(in_maps, **kwargs):
    from concourse.bass_utils import run_bass_kernel_spmd

    nc = _build_program()
    return run_bass_kernel_spmd(nc, in_maps, core_ids=list(range(_NCORES)), **kwargs)


def kernel(**inputs):
    res = _run(_shard_inputs(inputs["x1"], inputs["x2"]))
    return _gather(res.results)
